# revision 1
# baseline (speedup 1.0000x reference)
"""2-layer GAT (PyG GATConv heads=1) on 8 TRN2 cores.

Design:
- Nodes sharded by dst across cores (SHARD each, padded to SPAD=mult of 128).
- Per layer a row-major fp16 "table" of [h(64) | a_s | a_d | pad] rows (128 f16
  cols = 256B) lives in DRAM; per-edge source rows are fetched with dma_gather.
- Edges grouped per 128-dst block into 5 sections: 4 src-quarter sections
  (int16 gather indices < 32768) + 1 self-loop section (own-shard rows).
- Attention: e = leaky(a_s[src]+a_d[dst]); p = exp(e - SHIFT + mask);
  a_d per edge recovered via one-hot S (is_equal vs iota matrix) times a
  broadcast a_d row matrix, reduced along dst.
- Aggregation: PSUM accumulation of S_g^T @ [p*h | p] over the block's edge
  groups; normalize by the ones-column, add bias (+relu for layer 1).
- Layer-2 table built per block (transpose + matmul with W2_ext), shards
  AllGathered into the full table.
"""
import math
from contextlib import ExitStack

import numpy as np

import concourse.bacc as bacc
import concourse.bass as bass
import concourse.mybir as mybir
import concourse.tile as tile

F16 = mybir.dt.float16
F32 = mybir.dt.float32
I16 = mybir.dt.int16
U32 = mybir.dt.uint32

TROW = 128          # table row width in f16 elems (256B)
HID = 64
IN_DIM = 128
NEG = 0.2
SHIFT = 8.0
MASK_PAD = -60.0


class Cfg:
    def __init__(self, n_nodes, n_cores, in_dim=IN_DIM, hid=HID):
        assert n_nodes % n_cores == 0
        self.N = n_nodes
        self.C = n_cores
        self.SHARD = n_nodes // n_cores
        self.SPAD = ((self.SHARD + 127) // 128) * 128
        self.NB = self.SPAD // 128
        self.NQ = 4 if n_cores >= 4 else n_cores  # quarter count
        assert n_cores % self.NQ == 0
        self.QUARTER = (n_cores // self.NQ) * self.SPAD   # rows per quarter tensor
        self.in_dim = in_dim
        self.hid = hid
        # caps filled by preprocess
        self.caps = None
        self.NSLOT = None
        self.NG = None

    def set_caps(self, caps):
        # caps: per-quarter slot caps (multiples of 128) + selfloop section 128
        self.caps = list(caps) + [128]
        self.NSLOT = sum(self.caps)
        self.NG = self.NSLOT // 128
        self.secoff = np.cumsum([0] + self.caps).tolist()


def preprocess(cfg, edge_index):
    """Build per-core gather/one-hot tables from edge_index (+implicit self loops)."""
    src = np.asarray(edge_index[0], dtype=np.int64)
    dst = np.asarray(edge_index[1], dtype=np.int64)
    C, SHARD, SPAD, NB, NQ = cfg.C, cfg.SHARD, cfg.SPAD, cfg.NB, cfg.NQ

    # shard-padded global row of each node
    rows = (src // SHARD) * SPAD + (src % SHARD)
    q_of = rows // cfg.QUARTER
    inq = rows % cfg.QUARTER

    core_of = dst // SHARD
    block_of = (dst % SHARD) // 128
    doff_of = (dst % SHARD) % 128

    # bucket edges per (core, block, quarter)
    order = np.lexsort((dst, q_of, block_of, core_of))
    src_s, q_s, inq_s = src[order], q_of[order], inq[order]
    core_s, block_s, doff_s = core_of[order], block_of[order], doff_of[order]
    key = ((core_s * NB) + block_s) * NQ + q_s
    nbins = C * NB * NQ
    counts = np.bincount(key, minlength=nbins).reshape(C, NB, NQ)
    caps_q = counts.max(axis=(0, 1))
    caps = [int(math.ceil(c / 128) * 128) for c in caps_q]
    cfg.set_caps(caps)
    NSLOT, NG = cfg.NSLOT, cfg.NG

    idx_img = np.zeros((C, NB, 128, NSLOT // 16), dtype=np.int16)
    doff_img = np.full((C, NB, 128, NG), 999.0, dtype=np.float16)
    mask_img = np.full((C, NB, 128, NG), MASK_PAD, dtype=np.float16)
    cnt_img = np.zeros((C, 1, NB * NQ), dtype=np.uint32)

    starts = np.zeros(nbins + 1, dtype=np.int64)
    np.cumsum(counts.reshape(-1), out=starts[1:])

    for c in range(C):
        for b in range(NB):
            for q in range(NQ):
                k = ((c * NB) + b) * NQ + q
                n = counts[c, b, q]
                cnt_img[c, 0, b * NQ + q] = n
                if n == 0:
                    continue
                sl = slice(starts[k], starts[k] + n)
                inqs = inq_s[sl].astype(np.int16)
                doffs = doff_s[sl].astype(np.float16)
                so = cfg.secoff[q]
                # idx wrapped: idx i -> [i%16, (so+i)//16]
                i = np.arange(n)
                idx_img[c, b, i % 16, (so + i) // 16] = inqs
                # grid slot of idx i: partition (so+i)%128, group (so+i)//128
                doff_img[c, b, (so + i) % 128, (so + i) // 128] = doffs
                mask_img[c, b, (so + i) % 128, (so + i) // 128] = -SHIFT
            # self-loop section
            so = cfg.secoff[NQ]
            i = np.arange(128)
            loc = b * 128 + i
            idx_img[c, b, i % 16, (so + i) // 16] = loc.astype(np.int16)
            doff_img[c, b, (so + i) % 128, (so + i) // 128] = i.astype(np.float16)
            mask_img[c, b, (so + i) % 128, (so + i) // 128] = -SHIFT
    # replicate idx wrap across the 8 q7 cores (16-partition groups)
    idx_img = np.tile(idx_img[:, :, :16, :], (1, 1, 8, 1))
    # pack [idx | doff | mask] along the free dim as one int16 image
    pack = np.concatenate([
        idx_img.view(np.int16),
        doff_img.view(np.int16),
        mask_img.view(np.int16),
    ], axis=3)
    return pack, cnt_img


def host_tables(cfg, x, W1, as1, ad1, b1, W2, as2, ad2, b2, Wl, bl):
    C, SHARD, SPAD = cfg.C, cfg.SHARD, cfg.SPAD
    N, D = x.shape
    # xt_ext per core: [in_dim, NQ*QUARTER + SPAD]
    xpad = np.zeros((C * SPAD, D), dtype=np.float32)
    for c in range(C):
        xpad[c * SPAD:c * SPAD + SHARD] = x[c * SHARD:(c + 1) * SHARD]
    xt_quarters = xpad.T.astype(np.float16)  # [D, C*SPAD]
    xt_ext = []
    for c in range(C):
        own = xpad[c * SPAD:(c + 1) * SPAD].T.astype(np.float16)
        xt_ext.append(np.concatenate([xt_quarters, own], axis=1))
    w1e = np.concatenate([W1, (W1 @ as1)[:, None], (W1 @ ad1)[:, None]], axis=1).astype(np.float16)
    w2e = np.concatenate([W2, (W2 @ as2)[:, None], (W2 @ ad2)[:, None]], axis=1).astype(np.float16)
    b1rep = np.tile(b1[None, :], (128, 1)).astype(np.float32)
    b2rep = np.tile(b2[None, :], (128, 1)).astype(np.float32)
    blrep = np.tile(bl[None, :], (128, 1)).astype(np.float32)
    iota = np.tile(np.arange(128, dtype=np.float16)[None, :], (128, 1))
    ones_row = np.ones((1, 128), dtype=np.float16)
    ident = np.eye(128, dtype=np.float32)
    return xt_ext, dict(w1e=w1e, w2e=w2e, wl=Wl.astype(np.float32),
                        b1rep=b1rep, b2rep=b2rep, blrep=blrep,
                        iota=iota, ones_row=ones_row, ident=ident)


def build_nc(cfg, reps=1):
    C, SPAD, NB, NQ, NG, NSLOT = cfg.C, cfg.SPAD, cfg.NB, cfg.NQ, cfg.NG, cfg.NSLOT
    QUARTER = cfg.QUARTER
    XCOLS = NQ * QUARTER + SPAD
    D = cfg.in_dim
    H = cfg.hid

    nc = bacc.Bacc("TRN2", target_bir_lowering=False, debug=False, num_devices=C)
    inp = {}
    inp["xt"] = nc.dram_tensor("xt", [D, XCOLS], F16, kind="ExternalInput").ap()
    inp["w1e"] = nc.dram_tensor("w1e", [D, H + 2], F16, kind="ExternalInput").ap()
    inp["w2e"] = nc.dram_tensor("w2e", [H, H + 2], F16, kind="ExternalInput").ap()
    inp["wl"] = nc.dram_tensor("wl", [H, H], F32, kind="ExternalInput").ap()
    inp["b1rep"] = nc.dram_tensor("b1rep", [128, H], F32, kind="ExternalInput").ap()
    inp["b2rep"] = nc.dram_tensor("b2rep", [128, H], F32, kind="ExternalInput").ap()
    inp["blrep"] = nc.dram_tensor("blrep", [128, H], F32, kind="ExternalInput").ap()
    inp["iota"] = nc.dram_tensor("iota", [128, 128], F16, kind="ExternalInput").ap()
    inp["ones_row"] = nc.dram_tensor("ones_row", [1, 128], F16, kind="ExternalInput").ap()
    inp["ident"] = nc.dram_tensor("ident", [128, 128], F32, kind="ExternalInput").ap()
    PACKW = NSLOT // 16 + 2 * NG
    inp["pack_img"] = nc.dram_tensor("pack_img", [NB, 128, PACKW], I16, kind="ExternalInput").ap()
    y = nc.dram_tensor("y", [SPAD, H], F32, kind="ExternalOutput").ap()

    with tile.TileContext(nc) as tc, ExitStack() as ctx:
        dram = ctx.enter_context(tc.tile_pool(name="dram", bufs=1, space="DRAM"))
        t1q = [dram.tile([QUARTER, TROW], F16, tag=f"t1q{q}", name=f"t1q{q}") for q in range(NQ)]
        t1s = dram.tile([SPAD, TROW], F16, tag="t1s")
        ag = dram.tile([SPAD, TROW], F16, tag="ag")
        t2fs = [dram.tile([C * SPAD, TROW], F16, tag=f"t2f{r}", name=f"t2f{r}",
                          addr_space="Shared") for r in range(reps)]
        t2s = dram.tile([SPAD, TROW], F16, tag="t2s")

        consts = ctx.enter_context(tc.tile_pool(name="consts", bufs=1))
        w1e_t = consts.tile([D, H + 2], F16)
        nc.sync.dma_start(out=w1e_t[:], in_=inp["w1e"][:])
        w2e_t = consts.tile([H, H + 2], F16)
        nc.sync.dma_start(out=w2e_t[:], in_=inp["w2e"][:])
        wl_t = consts.tile([H, H], F32)
        nc.sync.dma_start(out=wl_t[:], in_=inp["wl"][:])
        b1_t = consts.tile([128, H], F32)
        nc.sync.dma_start(out=b1_t[:], in_=inp["b1rep"][:])
        b2_t = consts.tile([128, H], F32)
        nc.sync.dma_start(out=b2_t[:], in_=inp["b2rep"][:])
        bl_t = consts.tile([128, H], F32)
        nc.sync.dma_start(out=bl_t[:], in_=inp["blrep"][:])
        iota_t = consts.tile([128, 128], F16)
        nc.sync.dma_start(out=iota_t[:], in_=inp["iota"][:])
        ones_t = consts.tile([1, 128], F16)
        nc.sync.dma_start(out=ones_t[:], in_=inp["ones_row"][:])
        ident_t = consts.tile([128, 128], F32)
        nc.sync.dma_start(out=ident_t[:], in_=inp["ident"][:])

        # ---------------- Phase A: layer-1 table build ----------------
        bpool = ctx.enter_context(tc.tile_pool(name="build", bufs=4))
        bpsum = ctx.enter_context(tc.tile_pool(name="bpsum", bufs=2, space="PSUM"))
        ntile = XCOLS // 128
        qtiles = QUARTER // 128
        assert qtiles % 4 == 0 or qtiles < 4
        STEP = 4 if qtiles % 4 == 0 else 1
        def phase_build():
          for t0 in range(0, ntile, STEP):
              step = min(STEP, ntile - t0)
              xt_t = bpool.tile([D, 128 * STEP], F16, tag="xt", name="xt_t")
              nc.sync.dma_start(out=xt_t[:, 0:128 * step], in_=inp["xt"][:, t0 * 128:(t0 + step) * 128])
              trow = bpool.tile([128, STEP * TROW], F16, tag="trow", name="trow_t")
              for j in range(step):
                  ps = bpsum.tile([128, H + 2], F32, tag="bps")
                  nc.tensor.matmul(out=ps[:], lhsT=xt_t[:, j * 128:(j + 1) * 128], rhs=w1e_t[:],
                                   start=True, stop=True)
                  nc.vector.tensor_copy(out=trow[:, j * TROW:j * TROW + H + 2], in_=ps[:])
                  nc.vector.memset(trow[:, j * TROW + H + 2:(j + 1) * TROW], 0.0)
              t = t0
              if t < NQ * qtiles:
                  dest = t1q[t // qtiles][(t % qtiles) * 128:(t % qtiles) * 128 + 128 * step, :]
              else:
                  tt = t - NQ * qtiles
                  dest = t1s[tt * 128:tt * 128 + 128 * step, :]
              dest_v = dest.rearrange("(s p) e -> p s e", p=128)
              in_v = trow[:, 0:step * TROW].rearrange("p (s e) -> p s e", e=TROW)
              nc.sync.dma_start(out=dest_v, in_=in_v)

        # ---------------- Block processing ----------------
        pool = ctx.enter_context(tc.tile_pool(name="blk", bufs=3))
        spool = ctx.enter_context(tc.tile_pool(name="sall", bufs=2))
        psum = ctx.enter_context(tc.tile_pool(name="psum", bufs=2, space="PSUM"))
        psum2 = ctx.enter_context(tc.tile_pool(name="psum2", bufs=2, space="PSUM"))

        def process_block(b, tq, ts, layer):
            PACKW = NSLOT // 16 + 2 * NG
            pack_t = pool.tile([128, PACKW], I16, tag="pack")
            nc.sync.dma_start(out=pack_t[:], in_=inp["pack_img"][b])
            idx_t = pack_t[:, 0:NSLOT // 16]
            doff_t = pack_t[:, NSLOT // 16:NSLOT // 16 + NG].bitcast(F16)
            msk_t = pack_t[:, NSLOT // 16 + NG:NSLOT // 16 + 2 * NG].bitcast(F16)
            # a_d row of the block's dsts from own-shard table col 65
            adrow = pool.tile([1, 128], F16, tag="adrow")
            ts_flat = ts[:].rearrange("r c -> (r c)")
            col = bass.AP(ts_flat.tensor, ts_flat.offset + (b * 128 * TROW + H + 1),
                          [[0, 1], [TROW, 128]])
            nc.sync.dma_start(out=adrow[:], in_=col)

            hx = pool.tile([128, NG * TROW], F16, tag="hx")
            hx3 = hx[:].rearrange("p (g e) -> p g e", e=TROW)
            for q in range(NQ):
                so, cap = cfg.secoff[q], cfg.caps[q]
                nc.gpsimd.dma_gather(
                    out_ap=hx3[:, so // 128:(so + cap) // 128, :],
                    in_ap=tq[q][:],
                    idxs_ap=idx_t[:, so // 16:(so + cap) // 16],
                    num_idxs=cap, num_idxs_reg=cap,
                    elem_size=TROW, single_packet=False)
            # self-loop section: own-shard rows are sequential -> plain DMA
            so4 = cfg.secoff[NQ]
            nc.sync.dma_start(
                out=hx3[:, so4 // 128, :],
                in_=ts[b * 128:(b + 1) * 128, :])

            # a_d replicated matrix [128,128] via ones x adrow
            adps = psum2.tile([128, 128], F32, tag="scr")
            nc.tensor.matmul(out=adps[:], lhsT=ones_t[:], rhs=adrow[:], start=True, stop=True)
            adrep = pool.tile([128, 128], F16, tag="adrep")
            nc.vector.tensor_copy(out=adrep[:], in_=adps[:])

            # one-hot S (f16): [128, NG, 128]
            sall = spool.tile([128, NG * 128], F16, tag="sall")
            s3 = sall[:].rearrange("p (g d) -> p g d", d=128)
            d_ap = doff_t
            d_b = bass.AP(d_ap.tensor, d_ap.offset, [d_ap.ap[0], [1, NG], [0, 128]])
            i_ap = iota_t[:]
            i_b = bass.AP(i_ap.tensor, i_ap.offset, [i_ap.ap[0], [0, NG], [1, 128]])
            nc.vector.tensor_tensor(out=s3, in0=d_b, in1=i_b, op=mybir.AluOpType.is_equal)

            # a_dE [128, NG] f32 = reduce_d(S * adrep)
            prod = spool.tile([128, NG * 128], F16, tag="prod")
            a_ap = adrep[:]
            a_b = bass.AP(a_ap.tensor, a_ap.offset, [a_ap.ap[0], [0, NG], [1, 128]])
            nc.vector.tensor_tensor(out=prod[:].rearrange("p (g d) -> p g d", d=128),
                                    in0=s3, in1=a_b, op=mybir.AluOpType.mult)
            ade = pool.tile([128, NG], F32, tag="ade")
            nc.vector.tensor_reduce(out=ade[:], in_=prod[:].rearrange("p (g d) -> p g d", d=128),
                                    axis=mybir.AxisListType.X, op=mybir.AluOpType.add)

            # e pipeline [128, NG]
            ase = pool.tile([128, NG], F32, tag="ase")
            nc.vector.tensor_copy(out=ase[:], in_=hx3[:, :, H:H + 1].rearrange("p g e -> p (g e)"))
            e_t = pool.tile([128, NG], F32, tag="e")
            nc.vector.tensor_add(out=e_t[:], in0=ase[:], in1=ade[:])
            nc.vector.scalar_tensor_tensor(out=e_t[:], in0=e_t[:], scalar=NEG, in1=e_t[:],
                                           op0=mybir.AluOpType.mult, op1=mybir.AluOpType.max)
            nc.vector.tensor_add(out=e_t[:], in0=e_t[:], in1=msk_t)
            p_t = pool.tile([128, NG], F32, tag="p")
            nc.scalar.activation(out=p_t[:], in_=e_t[:], func=mybir.ActivationFunctionType.Exp)

            # Mp [128, NG, H+1] f16
            mp = pool.tile([128, NG * (H + 1)], F16, tag="mp")
            mp3 = mp[:].rearrange("p (g e) -> p g e", e=H + 1)
            p_ap = p_t[:]
            p_b = bass.AP(p_ap.tensor, p_ap.offset, [p_ap.ap[0], [1, NG], [0, H]])
            nc.vector.tensor_tensor(out=mp3[:, :, 0:H], in0=hx3[:, :, 0:H], in1=p_b,
                                    op=mybir.AluOpType.mult)
            nc.vector.tensor_copy(out=mp3[:, :, H:H + 1].rearrange("p g e -> p (g e)"), in_=p_t[:])

            # aggregation matmuls
            aps = psum.tile([128, H + 1], F32, tag="aps")
            for g in range(NG):
                nc.tensor.matmul(out=aps[:], lhsT=sall[:, g * 128:(g + 1) * 128],
                                 rhs=mp[:, g * (H + 1):(g + 1) * (H + 1)],
                                 start=(g == 0), stop=(g == NG - 1))

            # normalize
            den = pool.tile([128, 1], F32, tag="den")
            nc.vector.tensor_scalar_max(out=den[:], in0=aps[:, H:H + 1], scalar1=1e-30)
            rec = pool.tile([128, 1], F32, tag="rec")
            nc.vector.reciprocal(out=rec[:], in_=den[:])
            outn = pool.tile([128, H], F32, tag="outn")
            nc.vector.tensor_scalar_mul(out=outn[:], in0=aps[:, 0:H], scalar1=rec[:])

            if layer == 1:
                h2 = pool.tile([128, H], F32, tag="h2")
                nc.vector.tensor_tensor(out=h2[:], in0=outn[:], in1=b1_t[:], op=mybir.AluOpType.add)
                nc.vector.tensor_scalar_max(out=h2[:], in0=h2[:], scalar1=0.0)
                # transpose h2 -> [H, 128]
                tps = psum2.tile([128, 128], F32, tag="scr")
                nc.tensor.transpose(out=tps[0:H, :], in_=h2[:], identity=ident_t[:])
                h2t = pool.tile([H, 128], F16, tag="h2t")
                nc.vector.tensor_copy(out=h2t[:], in_=tps[0:H, :])
                ps3t = psum2.tile([128, 128], F32, tag="scr", name="ps3t")
                ps3 = ps3t[:, 0:H + 2]
                nc.tensor.matmul(out=ps3[:], lhsT=h2t[:], rhs=w2e_t[:], start=True, stop=True)
                trow = pool.tile([128, TROW], F16, tag="trow2")
                nc.vector.tensor_copy(out=trow[:, 0:H + 2], in_=ps3[:])
                nc.vector.memset(trow[:, H + 2:TROW], 0.0)
                nc.sync.dma_start(out=ag[b * 128:(b + 1) * 128, :], in_=trow[:])
                nc.sync.dma_start(out=t2s[b * 128:(b + 1) * 128, :], in_=trow[:])
            else:
                o2 = pool.tile([128, H], F32, tag="o2")
                nc.vector.tensor_tensor(out=o2[:], in0=outn[:], in1=b2_t[:], op=mybir.AluOpType.add)
                tps = psum2.tile([128, 128], F32, tag="scr")
                nc.tensor.transpose(out=tps[0:H, :], in_=o2[:], identity=ident_t[:])
                o2t = pool.tile([H, 128], F32, tag="o2t")
                nc.vector.tensor_copy(out=o2t[:], in_=tps[0:H, :])
                psyt = psum2.tile([128, 128], F32, tag="scr", name="psyt")
                psy = psyt[:, 0:H]
                nc.tensor.matmul(out=psy[:], lhsT=o2t[:], rhs=wl_t[:], start=True, stop=True)
                yt = pool.tile([128, H], F32, tag="yt")
                nc.vector.tensor_tensor(out=yt[:], in0=psy[:], in1=bl_t[:], op=mybir.AluOpType.add)
                nc.sync.dma_start(out=y[b * 128:(b + 1) * 128, :], in_=yt[:])

        for _rep in range(reps):
            t2f = t2fs[_rep]
            phase_build()
            for b in range(NB):
                process_block(b, t1q, t1s, layer=1)
            nc.gpsimd.collective_compute(
                "AllGather", mybir.AluOpType.bypass,
                replica_groups=[list(range(C))],
                ins=[ag[:]], outs=[t2f[:]])
            t2q = [t2f[q * QUARTER:(q + 1) * QUARTER, :] for q in range(NQ)]
            for b in range(NB):
                process_block(b, t2q, t2s, layer=2)

    nc.compile()
    return nc, inp


def run_gat(cfg, x, edge_index, params, run_fn):
    """params: dict with W1, att_src1, ... as numpy. run_fn(nc, in_maps) -> results."""
    pack_img, cnt_img = preprocess(cfg, edge_index)
    xt_ext, tabs = host_tables(cfg, x, params["W1"], params["att_src1"], params["att_dst1"],
                               params["b1"], params["W2"], params["att_src2"], params["att_dst2"],
                               params["b2"], params["Wl"], params["bl"])
    nc, _ = build_nc(cfg)
    in_maps = []
    for c in range(cfg.C):
        m = dict(tabs)
        m["xt"] = xt_ext[c]
        m["pack_img"] = pack_img[c]
        in_maps.append(m)
    results = run_fn(nc, in_maps)
    y = np.concatenate([results[c]["y"][:cfg.SHARD] for c in range(cfg.C)], axis=0)
    return y


# ----------------------------------------------------------------------------
# Harness entry point: kernel(**inputs) -> np.ndarray [100000, 64] float32
# ----------------------------------------------------------------------------
_CACHE = {}


def kernel(**inputs):
    from concourse.bass_utils import run_bass_kernel_spmd

    x = np.asarray(inputs["x"], dtype=np.float32)
    edge_index = np.asarray(inputs["edge_index"])
    params = dict(
        W1=np.asarray(inputs["W1"], np.float32), att_src1=np.asarray(inputs["att_src1"], np.float32),
        att_dst1=np.asarray(inputs["att_dst1"], np.float32), b1=np.asarray(inputs["b1"], np.float32),
        W2=np.asarray(inputs["W2"], np.float32), att_src2=np.asarray(inputs["att_src2"], np.float32),
        att_dst2=np.asarray(inputs["att_dst2"], np.float32), b2=np.asarray(inputs["b2"], np.float32),
        Wl=np.asarray(inputs["Wl"], np.float32), bl=np.asarray(inputs["bl"], np.float32),
    )
    n_nodes = x.shape[0]
    cfg = Cfg(n_nodes, 8)
    pack_img, cnt_img = preprocess(cfg, edge_index)
    key = (n_nodes, tuple(cfg.caps))
    if key not in _CACHE:
        _CACHE[key] = build_nc(cfg)
    nc, _ = _CACHE[key]
    xt_ext, tabs = host_tables(cfg, x, params["W1"], params["att_src1"], params["att_dst1"],
                               params["b1"], params["W2"], params["att_src2"], params["att_dst2"],
                               params["b2"], params["Wl"], params["bl"])
    in_maps = []
    for c in range(cfg.C):
        m = dict(tabs)
        m["xt"] = xt_ext[c]
        m["pack_img"] = pack_img[c]
        in_maps.append(m)
    res = run_bass_kernel_spmd(nc, in_maps, list(range(cfg.C)))
    y = np.concatenate([res.results[c]["y"][:cfg.SHARD] for c in range(cfg.C)], axis=0)
    return y.astype(np.float32)

